# revision 1
# baseline (speedup 1.0000x reference)
"""Trainium2 Bass kernel for nn_LocalClassifier (moe_routing).

Computation (reference):
    xr     = x.reshape(B, P, F)            # [32, 784, 2048] fp32
    Wg     = W[target]                     # [32, 2048]  per-batch gathered row
    logits = einsum('bpf,bf->bp', xr, Wg) + b[target][:, None]
    out    = sigmoid(logits).reshape(-1, 1, 1, 1)    # [25088, 1, 1, 1]

Strategy (8 NeuronCores, data parallel over B):
  - Host gathers the 4 W rows / bias values each core needs (the "routing"),
    shards B across the 8 cores (4 batches -> 3136 of the 25088 rows each),
    and pre-transposes each core's x shard to feature-major fp16 so the
    TensorEngine contracts over features (K=128 chunks on partitions).
    fp16 operands: PE streams single-pass and HBM traffic halves; PSUM
    accumulates fp32.
  - The 4 batches map to the PE array's four 32-wide column groups
    (tile_position (0, 32*b)), so each (chunk, half) quad of 128x1x392
    matmuls streams concurrently (~330 ns per quad).
  - Each DMA carries a group of k-chunks for ALL four batches
    ([128, n*4*784] fp16), so a group's matmul quads depend on exactly
    one DMA.  The two HWDGE rings (A=ACT/scalar, B=SP/sync) drain
    concurrently but share one serial descriptor-generation pipeline in
    global dispatch order, so groups are belt-paired A0 B0 A1 B1 ... and
    matmuls are emitted in the same order (PSUM accumulation commutes;
    start flag on the first emitted chunk, stop on the last).  wg goes
    through the independent SWDGE (gpsimd) generator and the tiny bg
    leads the sync queue, so belt A's first fat DMA deterministically
    wins the generation race and chunk 0 arrives first.  Groups are
    1-chunk at both ends (fast engine spin-up; tiny quad bursts after
    the final, possibly slow-engine-delayed, semaphores) with big 5-chunk
    groups mid-stream to minimize DMA-boundary bubbles.  All tiles stay
    resident (12.85 MB < SBUF).
  - Epilogue: per-half fused bias+sigmoid over PSUM partitions 0-96 (only
    rows {0,32,64,96} are consumed), strided single-packet DMAs write the
    [4, 392] fp32 halves; half 0's sigmoid+store overlap half 1's final
    matmuls, and the two stores dispatch from different queues (sync /
    scalar) so their DIRECT2Ds issue in parallel.
  - Memory-bound: 12.85 MB/core HBM reads at ~410 GB/s -> ~31 us stream.
"""

import sys

sys.path.insert(0, "/opt/trn_rl_repo")

import numpy as np

import concourse.bacc as bacc
import concourse.mybir as mybir
import concourse.tile as tile
from concourse.bass_utils import run_bass_kernel_spmd

B = 32      # batches
P = 784     # pixels per batch
F = 2048    # features
NCORES = 8
BPC = B // NCORES          # 4 batches per core
NK = F // 128              # 16 feature chunks of 128
NH = 2                     # split P into 2 matmul halves (PSUM bank = 512 fp32)
NHALF = P // NH            # 392
GELEM = 128 * BPC * P      # elements per chunk in the DRAM stream

# (belt, chunk indices) in emission order; belts alternate so the two
# HWDGE rings' fat DMAs pair up in the descriptor-gen FIFO.
# (belt, chunk indices, pixel-halves carried).  The final chunk k15 is
# split into its two pixel halves, one per ring: each ring's last DMA is
# half-size, the rings balance to 8.5/7.5 chunks, and each sigmoid's
# PSUM chain closes on its own ring's final semaphore.
GROUPS = [
    ("A", [0], (0, 1)),
    ("B", [1], (0, 1)),
    ("A", [2, 3, 4, 5, 6], (0, 1)),
    ("B", [7, 8, 9, 10, 11], (0, 1)),
    ("A", [12, 13], (0, 1)),
    ("B", [14], (0, 1)),
    ("A", [15], (0,)),
    ("B", [15], (1,)),
]

FP32 = mybir.dt.float32
FP16 = mybir.dt.float16

_NC_CACHE = {}


def _build_nc():
    nc = bacc.Bacc()
    total = sum(128 * len(ks) * BPC * len(hs) * NHALF for _, ks, hs in GROUPS)
    xt = nc.declare_dram_parameter("xt", [total], FP16, isOutput=False)
    wg = nc.declare_dram_parameter("wg", [128, BPC * NK], FP16, isOutput=False)
    bg = nc.declare_dram_parameter("bg", [128, 1], FP32, isOutput=False)
    out = nc.declare_dram_parameter("out", [BPC, P], FP32, isOutput=True)

    with tile.TileContext(nc) as tc:
        with (
            tc.tile_pool(name="xpool", bufs=1) as xpool,
            tc.tile_pool(name="cpool", bufs=1) as cpool,
            tc.tile_pool(name="psum", bufs=1, space="PSUM") as pp,
        ):
            wg_sb = cpool.tile([128, BPC * NK], FP16)
            bg_sb = cpool.tile([128, 1], FP32)
            out_sb = cpool.tile([128, P], FP32)

            # batch b accumulates in PSUM partition strip [32b, 32b+1)
            ps = [
                pp.tile([128, NHALF], FP32, name=f"ps{h}", tag=f"ps{h}")
                for h in range(NH)
            ]

            # wg via SWDGE (gpsimd): independent descriptor generator, so
            # it never perturbs the HWDGE A/B pairing.  The belts' first
            # groups are both single chunks, so whichever wins the
            # generation race costs at most ~1 us of (slack-covered) PE
            # start delay -- no stagger needed.
            nc.gpsimd.dma_start(out=wg_sb[:], in_=wg[:])

            tiles = []
            off = 0
            for g, (belt, ks, hs) in enumerate(GROUPS):
                wpix = len(hs) * NHALF
                w = len(ks) * BPC * wpix
                t = xpool.tile([128, w], FP16, name=f"x{g}", tag=f"x{g}")
                eng = nc.scalar if belt == "A" else nc.sync
                eng.dma_start(
                    out=t[:],
                    in_=xt[off : off + 128 * w].rearrange("(p f) -> p f", p=128),
                )
                tiles.append((t, ks, hs))
                off += 128 * w

            # bg is only needed at sigmoid time; trailing the sync queue
            # keeps it out of the stream's descriptor-generation window
            nc.sync.dma_start(out=bg_sb[:], in_=bg[:])

            # Every (b, h) PSUM chain sees its chunks in increasing-k
            # emission order, so start/stop key on the chunk index.
            for t, ks, hs in tiles:
                wpix = len(hs) * NHALF
                for c, k in enumerate(ks):
                    # h-major so each half's accumulation closes as early
                    # as possible in the final group
                    for hi, h in enumerate(hs):
                        for b in range(BPC):
                            col = b * NK + k
                            base = (c * BPC + b) * wpix + hi * NHALF
                            nc.tensor.matmul(
                                ps[h][32 * b : 32 * b + 1, :],
                                wg_sb[:, col : col + 1],
                                t[:, base : base + NHALF],
                                start=(k == 0),
                                stop=(k == NK - 1),
                                tile_position=(0, 32 * b),
                            )

            # one activation per half over partitions 0..96; lanes other
            # than {0,32,64,96} compute on garbage and are never read.
            # h0 first: the rings' final semaphores fire nearly together,
            # so sig0 starts immediately and sig1 overlaps the out0 store
            # (processing h1 first was measured to delay sig0 by ~0.6 us).
            for h in range(NH):
                nc.scalar.activation(
                    out_sb[0:97, h * NHALF : (h + 1) * NHALF],
                    ps[h][0:97, :],
                    mybir.ActivationFunctionType.Sigmoid,
                    bias=bg_sb[0:97, 0:1],
                    scale=1.0,
                )
                # both stores on sync: out0's dispatch overlaps sig1's
                # execution, so out1 dispatches with zero wait right
                # after; sync DIRECT2Ds also issue ~400 ns faster than
                # scalar ones (measured 609 vs 1038 ns).
                nc.sync.dma_start(
                    out=out[:, h * NHALF : (h + 1) * NHALF],
                    in_=out_sb[0:128:32, h * NHALF : (h + 1) * NHALF],
                    single_packet=True,
                )

    nc.finalize()
    return nc


def _get_nc():
    if "nc" not in _NC_CACHE:
        _NC_CACHE["nc"] = _build_nc()
    return _NC_CACHE["nc"]


def _make_in_maps(x, target, W, b):
    x = np.asarray(x, dtype=np.float32).reshape(B, P, F)
    target = np.asarray(target).astype(np.int64)
    W = np.asarray(W, dtype=np.float32)
    b = np.asarray(b, dtype=np.float32)

    Wg = W[target]          # [B, F]
    bg = b[target]          # [B]

    in_maps = []
    for m in range(NCORES):
        sl = slice(m * BPC, (m + 1) * BPC)
        # (b, p, k, e) -> (k, e, b, p), fp16
        xs = (
            x[sl]
            .astype(np.float16)
            .reshape(BPC, P, NK, 128)
            .transpose(2, 3, 0, 1)
        )  # [NK, 128, BPC, P]
        # per group: (k, e, b, p) -> (e, k, b, p) so each partition's group
        # data is one contiguous run; h-split groups carry one pixel half
        parts = []
        for _belt, ks, hs in GROUPS:
            blk = xs[ks]  # [n, 128, BPC, P]
            if hs != (0, 1):
                (h,) = hs
                blk = blk[:, :, :, h * NHALF : (h + 1) * NHALF]
            parts.append(blk.transpose(1, 0, 2, 3).reshape(-1))
        xtc = np.ascontiguousarray(np.concatenate(parts))
        # wg[p, b*NK + k] = Wg[b, k*128 + p]
        wgc = (
            Wg[sl]
            .reshape(BPC, NK, 128)
            .transpose(2, 0, 1)
            .reshape(128, BPC * NK)
            .astype(np.float16)
        )
        bgs = np.zeros((128, 1), np.float32)
        bgs[np.arange(BPC) * 32, 0] = bg[sl]
        in_maps.append({"xt": xtc, "wg": np.ascontiguousarray(wgc), "bg": bgs})
    return in_maps


def run(x, target, W, b, trace=False, **trace_kwargs):
    """Run on 8 cores; returns (full_output, BassKernelResults)."""
    nc = _get_nc()
    in_maps = _make_in_maps(x, target, W, b)
    res = run_bass_kernel_spmd(
        nc, in_maps, list(range(NCORES)), trace=trace, **trace_kwargs
    )
    outs = [res.results[i]["out"].reshape(-1) for i in range(NCORES)]
    full = np.concatenate(outs, axis=0).reshape(-1, 1, 1, 1).astype(np.float32)
    return full, res


def kernel(x, target, W, b):
    full, _ = run(x, target, W, b, trace=False)
    return full

